# revision 13
# baseline (speedup 1.0000x reference)
"""Block-diagonal (local) attention kernel for Trainium2, 8-core SPMD.

Problem: q, k, v = [8, 16, 4096, 128] fp32; block_size=128 local attention.
Per 128-token block: score = qb @ kb.T (no 1/sqrt(D) scaling), softmax over
keys, out = probs @ vb.  Blocks are independent -> shard batch across the 8
NeuronCores, no cross-device communication.

v3 design (fp32 baseline ~526 us, v2 ~228 us):
  - Host pre-transposes q and k into [d, w] block layout, so the device
    does NO PE transposes (the baseline spent 1/3 of its matmul-pipe time
    and a PSUM->SBUF copy per block on them).
  - 16-bit on the wire: q/k/v fp16, probs/unnormalized-out bf16 (these
    need fp32 exponent range: exp(s-25) reaches ~1e17), output fp16.
    Halves HBM traffic, 4x PE rate vs fp32.  Measured rel err ~1e-3 vs
    the 2e-2 gate.
  - One packed input DMA per head (qT|kT|v+ones contiguous, 3 MiB,
    16KB-contiguous per partition).
  - exp batched 8 blocks per ACTIVATE ([128,1024] over 2 PSUM banks):
    ACT cost is (172+free)/1.2 ns so batching amortizes the fixed cost.
  - v2 lesson: per-block normalize on DVE (tensor_scalar 512x344ns) PACED
    the whole pipeline (DMA only 86% active, bursts to 433 GB/s then
    starved).  v3 instead evicts the PV output unnormalized (plain
    copies, split ACT/DVE to balance engines), then does ONE batched
    reciprocal [128,32] and ONE broadcast tensor_mul [128,32x128] per
    chunk on the SBUF side.  DVE work per chunk: ~5 evict copies + 94ns
    recip + 4.3us multiply vs 32x344ns + 11x170ns before.

Per-block device work: 2 matmuls, 1/8 ACTIVATE, ~1/3 copy, 1/32 of
(recip + chunk-wide multiply).  Bottleneck: HBM DMA (~64 MiB/core).
"""

import numpy as np

import concourse.bass as bass
import concourse.tile as tile
from concourse import bacc, bass_utils, mybir

B = 8
H = 16
L = 4096
D = 128
W = 128            # attention block size
NB = L // W        # blocks per head (32)
N_CORES = 8
EXP_SHIFT = -25.0

CNB = 16           # blocks per chunk (= half a head)
N_CHUNKS = (H * NB) // CNB
QK_COLS = CNB * W          # 4096
V_COLS = CNB * (D + 1)     # 4128 (ones column baked in per block)
X_COLS = 2 * QK_COLS + V_COLS  # 12320
EG = 8             # blocks per exp group (two PSUM banks of scores)
PG = 3             # blocks per PV group (3*129 <= 512 psum cols)


def build_bass(num_devices: int = N_CORES) -> bass.Bass:
    f16 = mybir.dt.float16
    bf16 = mybir.dt.bfloat16
    f32 = mybir.dt.float32
    nc = bacc.Bacc(
        "TRN2", target_bir_lowering=False, debug=False, num_devices=num_devices
    )
    x = nc.dram_tensor("x", (N_CHUNKS * 128, X_COLS), f16, kind="ExternalInput").ap()
    o = nc.dram_tensor("out", (N_CHUNKS * 128, CNB * D), f16, kind="ExternalOutput").ap()

    with tile.TileContext(nc) as tc:
        with (
            tc.tile_pool(name="big", bufs=8) as big,
            tc.tile_pool(name="probs", bufs=8) as probs,
            tc.tile_pool(name="small", bufs=6) as small,
            tc.tile_pool(name="const", bufs=1) as const,
            tc.tile_pool(name="ps_s", bufs=2, space="PSUM") as ps_s,
            tc.tile_pool(name="ps_o", bufs=3, space="PSUM") as ps_o,
        ):
            exp_bias = const.tile([128, 1], f32)
            nc.gpsimd.memset(exp_bias, EXP_SHIFT)

            def score_exp(cc):
                """Input DMA + score matmuls + batched exp for chunk cc.
                Returns (xt, pTs) needed by the PV phase."""
                xt = big.tile([128, X_COLS], f16, tag="xt")
                nc.sync.dma_start(out=xt, in_=x[cc * 128 : (cc + 1) * 128])
                pTs = []
                for g in range(CNB // EG):
                    sT = ps_s.tile([128, EG * W], f32, tag="sT")
                    for i in range(EG):
                        n = g * EG + i
                        # sT[u, w] = k[u,:] . q[w,:]
                        nc.tensor.matmul(
                            sT[:, i * W : (i + 1) * W],
                            xt[:, QK_COLS + n * W : QK_COLS + (n + 1) * W],
                            xt[:, n * W : (n + 1) * W],
                        )
                    pT = probs.tile([128, EG * W], bf16, tag="pT")
                    nc.scalar.activation(
                        pT, sT, mybir.ActivationFunctionType.Exp,
                        bias=exp_bias, scale=1.0,
                    )
                    pTs.append(pT)
                return xt, pTs

            def pv_norm_out(cc, xt, pTs):
                """PV matmuls (PG blocks per PSUM bank), normalize straight
                out of PSUM (reciprocal of the denominator column + one
                broadcast tensor_mul PSUM->SBUF fp16), output DMA."""
                oh = big.tile([128, CNB, D], f16, tag="oh")
                for n0 in range(0, CNB, PG):
                    nn = min(PG, CNB - n0)
                    o_ps = ps_o.tile([128, PG, D + 1], f32, tag="o_ps")
                    for j in range(nn):
                        n = n0 + j
                        pT = pTs[n // EG][:, (n % EG) * W : (n % EG + 1) * W]
                        c0 = 2 * QK_COLS + n * (D + 1)
                        # out[w, 0:D] = probs @ vb ; out[w, D] = exp row sum
                        nc.tensor.matmul(
                            o_ps[:, j, :], pT, xt[:, c0 : c0 + D + 1]
                        )
                    r = small.tile([128, PG, 1], f32, tag="r")
                    nc.vector.reciprocal(
                        r[:, 0:nn, :], o_ps[:, 0:nn, D : D + 1]
                    )
                    nc.vector.tensor_mul(
                        oh[:, n0 : n0 + nn, :],
                        o_ps[:, 0:nn, 0:D],
                        r[:, 0:nn, :].broadcast_to([128, nn, D]),
                    )
                # output DMA trigger on the (otherwise idle) gpsimd queue:
                # its wait-for-oh must not block the sync queue's input
                # prefetch stream
                nc.gpsimd.dma_start(
                    out=o[cc * 128 : (cc + 1) * 128], in_=oh
                )

            # software-pipelined program order: chunk cc's scores are
            # emitted BEFORE chunk cc-1's PV phase, so the in-order PE
            # queue hides the exp latency behind the next chunk's score
            # matmuls instead of stalling (which also HAM-cools the PE)
            prev = None
            for cc in range(N_CHUNKS):
                cur = (cc, *score_exp(cc))
                if prev is not None:
                    pv_norm_out(*prev)
                prev = cur
            pv_norm_out(*prev)

    nc.compile()
    return nc


_nc_cache = None


def _get_nc() -> bass.Bass:
    global _nc_cache
    if _nc_cache is None:
        _nc_cache = build_bass()
    return _nc_cache


def _pack_inputs(q: np.ndarray, k: np.ndarray, v: np.ndarray) -> np.ndarray:
    """Pack one batch's q,k,v [H,L,D] fp32 into the device layout
    [N_CHUNKS*128, X_COLS] fp16: per chunk of CNB blocks,
    qT | kT | v-with-ones-column."""
    x = np.empty((N_CHUNKS, 128, X_COLS), dtype=np.float16)
    # (cc, p, w, d) -> (cc, d, p, w)
    x[:, :, :QK_COLS] = (
        q.reshape(N_CHUNKS, CNB, W, D).transpose(0, 3, 1, 2).reshape(N_CHUNKS, D, CNB * W)
    )
    x[:, :, QK_COLS : 2 * QK_COLS] = (
        k.reshape(N_CHUNKS, CNB, W, D).transpose(0, 3, 1, 2).reshape(N_CHUNKS, D, CNB * W)
    )
    xv = x[:, :, 2 * QK_COLS :].reshape(N_CHUNKS, 128, CNB, D + 1)
    # (cc, p, u, d) -> (cc, u, p, d)
    xv[:, :, :, :D] = v.reshape(N_CHUNKS, CNB, W, D).transpose(0, 2, 1, 3)
    xv[:, :, :, D] = 1.0
    return x.reshape(N_CHUNKS * 128, X_COLS)


def _prepare_in_maps(q, k, v):
    q = np.asarray(q, dtype=np.float32)
    k = np.asarray(k, dtype=np.float32)
    v = np.asarray(v, dtype=np.float32)
    assert q.shape == (B, H, L, D), q.shape
    return [{"x": _pack_inputs(q[b], k[b], v[b])} for b in range(B)]


def _unpack_out(o: np.ndarray) -> np.ndarray:
    """[N_CHUNKS*128, CNB*D] fp16 -> [H, L, D] fp32."""
    return (
        o.reshape(N_CHUNKS, W, CNB, D).transpose(0, 2, 1, 3).reshape(H, L, D)
    ).astype(np.float32)


def kernel(**inputs: np.ndarray) -> np.ndarray:
    nc = _get_nc()
    in_maps = _prepare_in_maps(inputs["q"], inputs["k"], inputs["v"])
    res = bass_utils.run_bass_kernel_spmd(nc, in_maps, core_ids=list(range(N_CORES)))
    return np.stack(
        [_unpack_out(np.asarray(res.results[b]["out"])) for b in range(B)], axis=0
    )


# revision 14
# speedup vs baseline: 1.0113x; 1.0113x over previous
"""Block-diagonal (local) attention kernel for Trainium2, 8-core SPMD.

Problem: q, k, v = [8, 16, 4096, 128] fp32; block_size=128 local attention.
Per 128-token block: score = qb @ kb.T (no 1/sqrt(D) scaling), softmax over
keys, out = probs @ vb.  Blocks are independent -> shard batch across the 8
NeuronCores, no cross-device communication.

v3 design (fp32 baseline ~526 us, v2 ~228 us):
  - Host pre-transposes q and k into [d, w] block layout, so the device
    does NO PE transposes (the baseline spent 1/3 of its matmul-pipe time
    and a PSUM->SBUF copy per block on them).
  - 16-bit on the wire: q/k/v fp16, probs/unnormalized-out bf16 (these
    need fp32 exponent range: exp(s-25) reaches ~1e17), output fp16.
    Halves HBM traffic, 4x PE rate vs fp32.  Measured rel err ~1e-3 vs
    the 2e-2 gate.
  - One packed input DMA per head (qT|kT|v+ones contiguous, 3 MiB,
    16KB-contiguous per partition).
  - exp batched 8 blocks per ACTIVATE ([128,1024] over 2 PSUM banks):
    ACT cost is (172+free)/1.2 ns so batching amortizes the fixed cost.
  - v2 lesson: per-block normalize on DVE (tensor_scalar 512x344ns) PACED
    the whole pipeline (DMA only 86% active, bursts to 433 GB/s then
    starved).  v3 instead evicts the PV output unnormalized (plain
    copies, split ACT/DVE to balance engines), then does ONE batched
    reciprocal [128,32] and ONE broadcast tensor_mul [128,32x128] per
    chunk on the SBUF side.  DVE work per chunk: ~5 evict copies + 94ns
    recip + 4.3us multiply vs 32x344ns + 11x170ns before.

Per-block device work: 2 matmuls, 1/8 ACTIVATE, ~1/3 copy, 1/32 of
(recip + chunk-wide multiply).  Bottleneck: HBM DMA (~64 MiB/core).
"""

import numpy as np

import concourse.bass as bass
import concourse.tile as tile
from concourse import bacc, bass_utils, mybir

B = 8
H = 16
L = 4096
D = 128
W = 128            # attention block size
NB = L // W        # blocks per head (32)
N_CORES = 8
EXP_SHIFT = -25.0

CNB = 16           # blocks per chunk (= half a head)
N_CHUNKS = (H * NB) // CNB
QK_COLS = CNB * W          # 4096
V_COLS = CNB * (D + 1)     # 4128 (ones column baked in per block)
X_COLS = 2 * QK_COLS + V_COLS  # 12320
EG = 4             # blocks per exp group (one PSUM bank of scores)
PG = 3             # blocks per PV group (3*129 <= 512 psum cols)


def build_bass(num_devices: int = N_CORES) -> bass.Bass:
    f16 = mybir.dt.float16
    bf16 = mybir.dt.bfloat16
    f32 = mybir.dt.float32
    nc = bacc.Bacc(
        "TRN2", target_bir_lowering=False, debug=False, num_devices=num_devices
    )
    x = nc.dram_tensor("x", (N_CHUNKS * 128, X_COLS), f16, kind="ExternalInput").ap()
    o = nc.dram_tensor("out", (N_CHUNKS * 128, CNB * D), f16, kind="ExternalOutput").ap()

    with tile.TileContext(nc) as tc:
        with (
            tc.tile_pool(name="big", bufs=8) as big,
            tc.tile_pool(name="probs", bufs=8) as probs,
            tc.tile_pool(name="small", bufs=6) as small,
            tc.tile_pool(name="const", bufs=1) as const,
            tc.tile_pool(name="ps_s", bufs=4, space="PSUM") as ps_s,
            tc.tile_pool(name="ps_o", bufs=4, space="PSUM") as ps_o,
        ):
            exp_bias = const.tile([128, 1], f32)
            nc.gpsimd.memset(exp_bias, EXP_SHIFT)

            def score_exp(cc):
                """Input DMA + score matmuls + batched exp for chunk cc.
                Returns (xt, pTs) needed by the PV phase."""
                xt = big.tile([128, X_COLS], f16, tag="xt")
                nc.sync.dma_start(out=xt, in_=x[cc * 128 : (cc + 1) * 128])
                pTs = []
                for g in range(CNB // EG):
                    sT = ps_s.tile([128, EG * W], f32, tag="sT")
                    for i in range(EG):
                        n = g * EG + i
                        # sT[u, w] = k[u,:] . q[w,:]
                        nc.tensor.matmul(
                            sT[:, i * W : (i + 1) * W],
                            xt[:, QK_COLS + n * W : QK_COLS + (n + 1) * W],
                            xt[:, n * W : (n + 1) * W],
                        )
                    pT = probs.tile([128, EG * W], bf16, tag="pT")
                    nc.scalar.activation(
                        pT, sT, mybir.ActivationFunctionType.Exp,
                        bias=exp_bias, scale=1.0,
                    )
                    pTs.append(pT)
                return xt, pTs

            def pv_norm_out(cc, xt, pTs):
                """PV matmuls (PG blocks per PSUM bank), normalize straight
                out of PSUM (reciprocal of the denominator column + one
                broadcast tensor_mul PSUM->SBUF fp16), output DMA."""
                oh = big.tile([128, CNB, D], f16, tag="oh")
                for n0 in range(0, CNB, PG):
                    nn = min(PG, CNB - n0)
                    o_ps = ps_o.tile([128, PG, D + 1], f32, tag="o_ps")
                    for j in range(nn):
                        n = n0 + j
                        pT = pTs[n // EG][:, (n % EG) * W : (n % EG + 1) * W]
                        c0 = 2 * QK_COLS + n * (D + 1)
                        # out[w, 0:D] = probs @ vb ; out[w, D] = exp row sum
                        nc.tensor.matmul(
                            o_ps[:, j, :], pT, xt[:, c0 : c0 + D + 1]
                        )
                    r = small.tile([128, PG, 1], f32, tag="r")
                    nc.vector.reciprocal(
                        r[:, 0:nn, :], o_ps[:, 0:nn, D : D + 1]
                    )
                    if n0 // PG in (1, 4):
                        # offload these groups to ACT as per-block fused
                        # scale-copies (scale is a per-partition vector)
                        for j in range(nn):
                            nc.scalar.mul(
                                oh[:, n0 + j, :],
                                o_ps[:, j, 0:D],
                                r[:, j, :],
                            )
                    else:
                        nc.vector.tensor_mul(
                            oh[:, n0 : n0 + nn, :],
                            o_ps[:, 0:nn, 0:D],
                            r[:, 0:nn, :].broadcast_to([128, nn, D]),
                        )
                # output DMA trigger on the (otherwise idle) gpsimd queue:
                # its wait-for-oh must not block the sync queue's input
                # prefetch stream
                nc.gpsimd.dma_start(
                    out=o[cc * 128 : (cc + 1) * 128], in_=oh
                )

            # software-pipelined program order: chunk cc's scores are
            # emitted BEFORE chunk cc-1's PV phase, so the in-order PE
            # queue hides the exp latency behind the next chunk's score
            # matmuls instead of stalling (which also HAM-cools the PE)
            prev = None
            for cc in range(N_CHUNKS):
                cur = (cc, *score_exp(cc))
                if prev is not None:
                    pv_norm_out(*prev)
                prev = cur
            pv_norm_out(*prev)

    nc.compile()
    return nc


_nc_cache = None


def _get_nc() -> bass.Bass:
    global _nc_cache
    if _nc_cache is None:
        _nc_cache = build_bass()
    return _nc_cache


def _pack_inputs(q: np.ndarray, k: np.ndarray, v: np.ndarray) -> np.ndarray:
    """Pack one batch's q,k,v [H,L,D] fp32 into the device layout
    [N_CHUNKS*128, X_COLS] fp16: per chunk of CNB blocks,
    qT | kT | v-with-ones-column."""
    x = np.empty((N_CHUNKS, 128, X_COLS), dtype=np.float16)
    # (cc, p, w, d) -> (cc, d, p, w)
    x[:, :, :QK_COLS] = (
        q.reshape(N_CHUNKS, CNB, W, D).transpose(0, 3, 1, 2).reshape(N_CHUNKS, D, CNB * W)
    )
    x[:, :, QK_COLS : 2 * QK_COLS] = (
        k.reshape(N_CHUNKS, CNB, W, D).transpose(0, 3, 1, 2).reshape(N_CHUNKS, D, CNB * W)
    )
    xv = x[:, :, 2 * QK_COLS :].reshape(N_CHUNKS, 128, CNB, D + 1)
    # (cc, p, u, d) -> (cc, u, p, d)
    xv[:, :, :, :D] = v.reshape(N_CHUNKS, CNB, W, D).transpose(0, 2, 1, 3)
    xv[:, :, :, D] = 1.0
    return x.reshape(N_CHUNKS * 128, X_COLS)


def _prepare_in_maps(q, k, v):
    q = np.asarray(q, dtype=np.float32)
    k = np.asarray(k, dtype=np.float32)
    v = np.asarray(v, dtype=np.float32)
    assert q.shape == (B, H, L, D), q.shape
    return [{"x": _pack_inputs(q[b], k[b], v[b])} for b in range(B)]


def _unpack_out(o: np.ndarray) -> np.ndarray:
    """[N_CHUNKS*128, CNB*D] fp16 -> [H, L, D] fp32."""
    return (
        o.reshape(N_CHUNKS, W, CNB, D).transpose(0, 2, 1, 3).reshape(H, L, D)
    ).astype(np.float32)


def kernel(**inputs: np.ndarray) -> np.ndarray:
    nc = _get_nc()
    in_maps = _prepare_in_maps(inputs["q"], inputs["k"], inputs["v"])
    res = bass_utils.run_bass_kernel_spmd(nc, in_maps, core_ids=list(range(N_CORES)))
    return np.stack(
        [_unpack_out(np.asarray(res.results[b]["out"])) for b in range(B)], axis=0
    )


# revision 15
# speedup vs baseline: 1.0142x; 1.0029x over previous
"""Block-diagonal (local) attention kernel for Trainium2, 8-core SPMD.

Problem: q, k, v = [8, 16, 4096, 128] fp32; block_size=128 local attention.
Per 128-token block: score = qb @ kb.T (no 1/sqrt(D) scaling), softmax over
keys, out = probs @ vb.  Blocks are independent -> shard batch across the 8
NeuronCores, no cross-device communication.

v3 design (fp32 baseline ~526 us, v2 ~228 us):
  - Host pre-transposes q and k into [d, w] block layout, so the device
    does NO PE transposes (the baseline spent 1/3 of its matmul-pipe time
    and a PSUM->SBUF copy per block on them).
  - 16-bit on the wire: q/k/v fp16, probs/unnormalized-out bf16 (these
    need fp32 exponent range: exp(s-25) reaches ~1e17), output fp16.
    Halves HBM traffic, 4x PE rate vs fp32.  Measured rel err ~1e-3 vs
    the 2e-2 gate.
  - One packed input DMA per head (qT|kT|v+ones contiguous, 3 MiB,
    16KB-contiguous per partition).
  - exp batched 8 blocks per ACTIVATE ([128,1024] over 2 PSUM banks):
    ACT cost is (172+free)/1.2 ns so batching amortizes the fixed cost.
  - v2 lesson: per-block normalize on DVE (tensor_scalar 512x344ns) PACED
    the whole pipeline (DMA only 86% active, bursts to 433 GB/s then
    starved).  v3 instead evicts the PV output unnormalized (plain
    copies, split ACT/DVE to balance engines), then does ONE batched
    reciprocal [128,32] and ONE broadcast tensor_mul [128,32x128] per
    chunk on the SBUF side.  DVE work per chunk: ~5 evict copies + 94ns
    recip + 4.3us multiply vs 32x344ns + 11x170ns before.

Per-block device work: 2 matmuls, 1/8 ACTIVATE, ~1/3 copy, 1/32 of
(recip + chunk-wide multiply).  Bottleneck: HBM DMA (~64 MiB/core).
"""

import numpy as np

import concourse.bass as bass
import concourse.tile as tile
from concourse import bacc, bass_utils, mybir

B = 8
H = 16
L = 4096
D = 128
W = 128            # attention block size
NB = L // W        # blocks per head (32)
N_CORES = 8
EXP_SHIFT = -25.0

CNB = 16           # blocks per chunk (= half a head)
N_CHUNKS = (H * NB) // CNB
QK_COLS = CNB * W          # 4096
V_COLS = CNB * (D + 1)     # 4128 (ones column baked in per block)
X_COLS = 2 * QK_COLS + V_COLS  # 12320
EG = 4             # blocks per exp group (one PSUM bank of scores)
PG = 3             # blocks per PV group (3*129 <= 512 psum cols)


def build_bass(num_devices: int = N_CORES) -> bass.Bass:
    f16 = mybir.dt.float16
    bf16 = mybir.dt.bfloat16
    f32 = mybir.dt.float32
    nc = bacc.Bacc(
        "TRN2", target_bir_lowering=False, debug=False, num_devices=num_devices
    )
    x = nc.dram_tensor("x", (N_CHUNKS * 128, X_COLS), f16, kind="ExternalInput").ap()
    o = nc.dram_tensor("out", (N_CHUNKS * 128, CNB * D), f16, kind="ExternalOutput").ap()

    with tile.TileContext(nc) as tc:
        with (
            tc.tile_pool(name="big", bufs=8) as big,
            tc.tile_pool(name="probs", bufs=8) as probs,
            tc.tile_pool(name="small", bufs=6) as small,
            tc.tile_pool(name="const", bufs=1) as const,
            tc.tile_pool(name="ps_s", bufs=4, space="PSUM") as ps_s,
            tc.tile_pool(name="ps_o", bufs=4, space="PSUM") as ps_o,
        ):
            exp_bias = const.tile([128, 1], f32)
            nc.gpsimd.memset(exp_bias, EXP_SHIFT)

            def score_exp(cc):
                """Input DMA + score matmuls + batched exp for chunk cc.
                Returns (xt, pTs) needed by the PV phase."""
                xt = big.tile([128, X_COLS], f16, tag="xt")
                nc.sync.dma_start(out=xt, in_=x[cc * 128 : (cc + 1) * 128])
                pTs = []
                for g in range(CNB // EG):
                    sT = ps_s.tile([128, EG * W], f32, tag="sT")
                    for i in range(EG):
                        n = g * EG + i
                        # sT[u, w] = k[u,:] . q[w,:]
                        nc.tensor.matmul(
                            sT[:, i * W : (i + 1) * W],
                            xt[:, QK_COLS + n * W : QK_COLS + (n + 1) * W],
                            xt[:, n * W : (n + 1) * W],
                        )
                    pT = probs.tile([128, EG * W], bf16, tag="pT")
                    nc.scalar.activation(
                        pT, sT, mybir.ActivationFunctionType.Exp,
                        bias=exp_bias, scale=1.0,
                    )
                    pTs.append(pT)
                return xt, pTs

            def pv_norm_out(cc, xt, pTs):
                """PV matmuls (PG blocks per PSUM bank), normalize straight
                out of PSUM (reciprocal of the denominator column + one
                broadcast tensor_mul PSUM->SBUF fp16), output DMA."""
                oh = big.tile([128, CNB, D], f16, tag="oh")
                for n0 in range(0, CNB, PG):
                    nn = min(PG, CNB - n0)
                    o_ps = ps_o.tile([128, PG, D + 1], f32, tag="o_ps")
                    for j in range(nn):
                        n = n0 + j
                        pT = pTs[n // EG][:, (n % EG) * W : (n % EG + 1) * W]
                        c0 = 2 * QK_COLS + n * (D + 1)
                        # out[w, 0:D] = probs @ vb ; out[w, D] = exp row sum
                        nc.tensor.matmul(
                            o_ps[:, j, :], pT, xt[:, c0 : c0 + D + 1]
                        )
                    r = small.tile([128, PG, 1], f32, tag="r")
                    nc.vector.reciprocal(
                        r[:, 0:nn, :], o_ps[:, 0:nn, D : D + 1]
                    )
                    nc.vector.tensor_mul(
                        oh[:, n0 : n0 + nn, :],
                        o_ps[:, 0:nn, 0:D],
                        r[:, 0:nn, :].broadcast_to([128, nn, D]),
                    )
                # output DMA trigger on the (otherwise idle) gpsimd queue:
                # its wait-for-oh must not block the sync queue's input
                # prefetch stream
                nc.gpsimd.dma_start(
                    out=o[cc * 128 : (cc + 1) * 128], in_=oh
                )

            # software-pipelined program order: chunk cc's scores are
            # emitted BEFORE chunk cc-1's PV phase, so the in-order PE
            # queue hides the exp latency behind the next chunk's score
            # matmuls instead of stalling (which also HAM-cools the PE)
            prev = None
            for cc in range(N_CHUNKS):
                cur = (cc, *score_exp(cc))
                if prev is not None:
                    pv_norm_out(*prev)
                prev = cur
            pv_norm_out(*prev)

    nc.compile()
    return nc


_nc_cache = None


def _get_nc() -> bass.Bass:
    global _nc_cache
    if _nc_cache is None:
        _nc_cache = build_bass()
    return _nc_cache


def _pack_inputs(q: np.ndarray, k: np.ndarray, v: np.ndarray) -> np.ndarray:
    """Pack one batch's q,k,v [H,L,D] fp32 into the device layout
    [N_CHUNKS*128, X_COLS] fp16: per chunk of CNB blocks,
    qT | kT | v-with-ones-column."""
    x = np.empty((N_CHUNKS, 128, X_COLS), dtype=np.float16)
    # (cc, p, w, d) -> (cc, d, p, w)
    x[:, :, :QK_COLS] = (
        q.reshape(N_CHUNKS, CNB, W, D).transpose(0, 3, 1, 2).reshape(N_CHUNKS, D, CNB * W)
    )
    x[:, :, QK_COLS : 2 * QK_COLS] = (
        k.reshape(N_CHUNKS, CNB, W, D).transpose(0, 3, 1, 2).reshape(N_CHUNKS, D, CNB * W)
    )
    xv = x[:, :, 2 * QK_COLS :].reshape(N_CHUNKS, 128, CNB, D + 1)
    # (cc, p, u, d) -> (cc, u, p, d)
    xv[:, :, :, :D] = v.reshape(N_CHUNKS, CNB, W, D).transpose(0, 2, 1, 3)
    xv[:, :, :, D] = 1.0
    return x.reshape(N_CHUNKS * 128, X_COLS)


def _prepare_in_maps(q, k, v):
    q = np.asarray(q, dtype=np.float32)
    k = np.asarray(k, dtype=np.float32)
    v = np.asarray(v, dtype=np.float32)
    assert q.shape == (B, H, L, D), q.shape
    return [{"x": _pack_inputs(q[b], k[b], v[b])} for b in range(B)]


def _unpack_out(o: np.ndarray) -> np.ndarray:
    """[N_CHUNKS*128, CNB*D] fp16 -> [H, L, D] fp32."""
    return (
        o.reshape(N_CHUNKS, W, CNB, D).transpose(0, 2, 1, 3).reshape(H, L, D)
    ).astype(np.float32)


def kernel(**inputs: np.ndarray) -> np.ndarray:
    nc = _get_nc()
    in_maps = _prepare_in_maps(inputs["q"], inputs["k"], inputs["v"])
    res = bass_utils.run_bass_kernel_spmd(nc, in_maps, core_ids=list(range(N_CORES)))
    return np.stack(
        [_unpack_out(np.asarray(res.results[b]["out"])) for b in range(B)], axis=0
    )


# revision 16
# speedup vs baseline: 1.0184x; 1.0042x over previous
"""Block-diagonal (local) attention kernel for Trainium2, 8-core SPMD.

Problem: q, k, v = [8, 16, 4096, 128] fp32; block_size=128 local attention.
Per 128-token block: score = qb @ kb.T (no 1/sqrt(D) scaling), softmax over
keys, out = probs @ vb.  Blocks are independent -> shard batch across the 8
NeuronCores, no cross-device communication.

Design log (fp32 baseline ~526us -> v2 228 -> v3/v4 ~202 -> this):
  - Host pre-transposes q,k into [d, w] block layout: no PE transposes.
  - 16-bit wire: q/k/v fp16, probs bf16 (needs fp32 exponent range:
    exp(s-25) reaches ~1e17), out fp16.  Halves HBM bytes, 4x PE rate.
  - Per half-head chunk (16 blocks): score matmuls into PSUM, exp batched
    8 blocks per ACTIVATE ([128,1024], amortizes ACT's 172-cycle fixed
    cost), PV matmuls 3 blocks per PSUM bank with a host-baked ones
    column in v producing the softmax denominator for free, then
    normalize STRAIGHT out of PSUM: reciprocal of the denominator column
    + broadcast tensor_mul -> fp16 output tile (no staging copies).
    One PV group per chunk normalizes on ACT (per-block scale-copy,
    scale = per-partition reciprocal vector) to balance ACT/DVE.
  - Input split in two DMAs: qk tile is released by the score matmuls
    (early), the small v tile is held until PV.  A combined tile made
    input prefetch wait on the previous chunk's LAST PV matmul, eroding
    the DMA lead until every chunk paid full transfer latency.
  - Output DMA triggers issue from the (otherwise idle) gpsimd queue:
    on the sync queue their wait-for-output blocked the input prefetch
    stream.  Software-pipelined program order (chunk cc scores emitted
    before chunk cc-1 PV) hides exp latency from the in-order PE queue.

Bottleneck: HBM DMA (~64 MiB/core, ~420 GB/s observed sustained).
"""

import numpy as np

import concourse.bass as bass
import concourse.tile as tile
from concourse import bacc, bass_utils, mybir

B = 8
H = 16
L = 4096
D = 128
W = 128            # attention block size
NB = L // W        # blocks per head (32)
N_CORES = 8
EXP_SHIFT = -25.0

CNB = 16           # blocks per chunk (= half a head)
N_CHUNKS = (H * NB) // CNB
QK_COLS = 2 * CNB * W      # 4096: qT | kT
V_COLS = CNB * (D + 1)     # 2064: v with ones column baked in per block
EG = 8             # blocks per exp group (two PSUM banks of scores)
PG = 3             # blocks per PV group (3*129 <= 512 psum cols)
ACT_GROUP = 2      # PV group whose normalize runs on ACT instead of DVE


def build_bass(num_devices: int = N_CORES) -> bass.Bass:
    f16 = mybir.dt.float16
    bf16 = mybir.dt.bfloat16
    f32 = mybir.dt.float32
    nc = bacc.Bacc(
        "TRN2", target_bir_lowering=False, debug=False, num_devices=num_devices
    )
    xqk = nc.dram_tensor(
        "xqk", (N_CHUNKS * 128, QK_COLS), f16, kind="ExternalInput"
    ).ap()
    xv = nc.dram_tensor(
        "xv", (N_CHUNKS * 128, V_COLS), f16, kind="ExternalInput"
    ).ap()
    o = nc.dram_tensor(
        "out", (N_CHUNKS * 128, CNB * D), f16, kind="ExternalOutput"
    ).ap()

    with tile.TileContext(nc) as tc:
        with (
            tc.tile_pool(name="pqk", bufs=8) as pqk,
            tc.tile_pool(name="pv", bufs=12) as pv,
            tc.tile_pool(name="po", bufs=6) as po,
            tc.tile_pool(name="probs", bufs=8) as probs,
            tc.tile_pool(name="small", bufs=8) as small,
            tc.tile_pool(name="const", bufs=1) as const,
            tc.tile_pool(name="ps_s", bufs=2, space="PSUM") as ps_s,
            tc.tile_pool(name="ps_o", bufs=4, space="PSUM") as ps_o,
        ):
            exp_bias = const.tile([128, 1], f32)
            nc.gpsimd.memset(exp_bias, EXP_SHIFT)

            def score_exp(cc):
                """Input DMAs + score matmuls + batched exp for chunk cc.
                Returns (vt, pTs) needed by the PV phase."""
                qk = pqk.tile([128, QK_COLS], f16, tag="qk")
                nc.sync.dma_start(out=qk, in_=xqk[cc * 128 : (cc + 1) * 128])
                vt = pv.tile([128, V_COLS], f16, tag="vt")
                nc.sync.dma_start(out=vt, in_=xv[cc * 128 : (cc + 1) * 128])
                pTs = []
                for g in range(CNB // EG):
                    sT = ps_s.tile([128, EG * W], f32, tag="sT")
                    for i in range(EG):
                        n = g * EG + i
                        # sT[u, w] = k[u,:] . q[w,:]
                        nc.tensor.matmul(
                            sT[:, i * W : (i + 1) * W],
                            qk[:, (CNB + n) * W : (CNB + n + 1) * W],
                            qk[:, n * W : (n + 1) * W],
                        )
                    pT = probs.tile([128, EG * W], bf16, tag="pT")
                    nc.scalar.activation(
                        pT, sT, mybir.ActivationFunctionType.Exp,
                        bias=exp_bias, scale=1.0,
                    )
                    pTs.append(pT)
                return vt, pTs

            def pv_norm_out(cc, vt, pTs):
                """PV matmuls (PG blocks per PSUM bank), normalize straight
                out of PSUM (reciprocal of the denominator column, then a
                broadcast tensor_mul on DVE -- or per-block ACT scale-copies
                for one group per chunk to balance the queues), output DMA."""
                oh = po.tile([128, CNB, D], f16, tag="oh")
                for gi, n0 in enumerate(range(0, CNB, PG)):
                    nn = min(PG, CNB - n0)
                    o_ps = ps_o.tile([128, PG, D + 1], f32, tag="o_ps")
                    for j in range(nn):
                        n = n0 + j
                        pT = pTs[n // EG][:, (n % EG) * W : (n % EG + 1) * W]
                        # out[w, 0:D] = probs @ vb ; out[w, D] = exp row sum
                        nc.tensor.matmul(
                            o_ps[:, j, :],
                            pT,
                            vt[:, n * (D + 1) : (n + 1) * (D + 1)],
                        )
                    r = small.tile([128, PG, 1], f32, tag="r")
                    nc.vector.reciprocal(
                        r[:, 0:nn, :], o_ps[:, 0:nn, D : D + 1]
                    )
                    if gi == ACT_GROUP:
                        # per-block fused scale-copy on ACT (scale is a
                        # per-partition vector) to offload DVE
                        for j in range(nn):
                            nc.scalar.mul(
                                oh[:, n0 + j, :], o_ps[:, j, 0:D], r[:, j, :]
                            )
                    else:
                        nc.vector.tensor_mul(
                            oh[:, n0 : n0 + nn, :],
                            o_ps[:, 0:nn, 0:D],
                            r[:, 0:nn, :].broadcast_to([128, nn, D]),
                        )
                nc.gpsimd.dma_start(
                    out=o[cc * 128 : (cc + 1) * 128], in_=oh
                )

            # software pipelining: emit chunk cc's scores before chunk
            # cc-1's PV phase
            prev = None
            for cc in range(N_CHUNKS):
                cur = (cc, *score_exp(cc))
                if prev is not None:
                    pv_norm_out(*prev)
                prev = cur
            pv_norm_out(*prev)

    nc.compile()
    return nc


_nc_cache = None


def _get_nc() -> bass.Bass:
    global _nc_cache
    if _nc_cache is None:
        _nc_cache = build_bass()
    return _nc_cache


def _pack_inputs(q, k, v):
    """Pack one batch's q,k,v [H,L,D] fp32 into device layouts:
    xqk [N_CHUNKS*128, QK_COLS] (qT | kT per chunk) and
    xv [N_CHUNKS*128, V_COLS] (v with ones column per block), both fp16."""
    xqk = np.empty((N_CHUNKS, 128, QK_COLS), dtype=np.float16)
    half = CNB * W
    # (cc, p, w, d) -> (cc, d, p, w)
    xqk[:, :, :half] = (
        q.reshape(N_CHUNKS, CNB, W, D).transpose(0, 3, 1, 2).reshape(N_CHUNKS, D, half)
    )
    xqk[:, :, half:] = (
        k.reshape(N_CHUNKS, CNB, W, D).transpose(0, 3, 1, 2).reshape(N_CHUNKS, D, half)
    )
    xv = np.empty((N_CHUNKS, 128, CNB, D + 1), dtype=np.float16)
    # (cc, p, u, d) -> (cc, u, p, d)
    xv[:, :, :, :D] = v.reshape(N_CHUNKS, CNB, W, D).transpose(0, 2, 1, 3)
    xv[:, :, :, D] = 1.0
    return (
        xqk.reshape(N_CHUNKS * 128, QK_COLS),
        xv.reshape(N_CHUNKS * 128, V_COLS),
    )


def _prepare_in_maps(q, k, v):
    q = np.asarray(q, dtype=np.float32)
    k = np.asarray(k, dtype=np.float32)
    v = np.asarray(v, dtype=np.float32)
    assert q.shape == (B, H, L, D), q.shape
    maps = []
    for b in range(B):
        xqk, xv = _pack_inputs(q[b], k[b], v[b])
        maps.append({"xqk": xqk, "xv": xv})
    return maps


def _unpack_out(o: np.ndarray) -> np.ndarray:
    """[N_CHUNKS*128, CNB*D] fp16 -> [H, L, D] fp32."""
    return (
        o.reshape(N_CHUNKS, W, CNB, D).transpose(0, 2, 1, 3).reshape(H, L, D)
    ).astype(np.float32)


def kernel(**inputs: np.ndarray) -> np.ndarray:
    nc = _get_nc()
    in_maps = _prepare_in_maps(inputs["q"], inputs["k"], inputs["v"])
    res = bass_utils.run_bass_kernel_spmd(nc, in_maps, core_ids=list(range(N_CORES)))
    return np.stack(
        [_unpack_out(np.asarray(res.results[b]["out"])) for b in range(B)], axis=0
    )


# revision 17
# speedup vs baseline: 1.0834x; 1.0638x over previous
"""Block-diagonal (local) attention kernel for Trainium2, 8-core SPMD.

Problem: q, k, v = [8, 16, 4096, 128] fp32; block_size=128 local attention.
Per 128-token block: score = qb @ kb.T (no 1/sqrt(D) scaling), softmax over
keys, out = probs @ vb.  Blocks are independent -> shard batch across the 8
NeuronCores, no cross-device communication.

Design log (fp32 baseline ~526us -> v2 228 -> v3/v4 ~202 -> this):
  - Host pre-transposes q,k into [d, w] block layout: no PE transposes.
  - 16-bit wire: q/k/v fp16, probs bf16 (needs fp32 exponent range:
    exp(s-25) reaches ~1e17), out fp16.  Halves HBM bytes, 4x PE rate.
  - Per half-head chunk (16 blocks): score matmuls into PSUM, exp batched
    8 blocks per ACTIVATE ([128,1024], amortizes ACT's 172-cycle fixed
    cost), PV matmuls 3 blocks per PSUM bank with a host-baked ones
    column in v producing the softmax denominator for free, then
    normalize STRAIGHT out of PSUM: reciprocal of the denominator column
    + broadcast tensor_mul -> fp16 output tile (no staging copies).
    One PV group per chunk normalizes on ACT (per-block scale-copy,
    scale = per-partition reciprocal vector) to balance ACT/DVE.
  - Input split in two DMAs: qk tile is released by the score matmuls
    (early), the small v tile is held until PV.  A combined tile made
    input prefetch wait on the previous chunk's LAST PV matmul, eroding
    the DMA lead until every chunk paid full transfer latency.
  - Output DMA triggers issue from the (otherwise idle) gpsimd queue:
    on the sync queue their wait-for-output blocked the input prefetch
    stream.  Software-pipelined program order (chunk cc scores emitted
    before chunk cc-1 PV) hides exp latency from the in-order PE queue.

Bottleneck: HBM DMA (~64 MiB/core, ~420 GB/s observed sustained).
"""

import numpy as np

import concourse.bass as bass
import concourse.tile as tile
from concourse import bacc, bass_utils, mybir

B = 8
H = 16
L = 4096
D = 128
W = 128            # attention block size
NB = L // W        # blocks per head (32)
N_CORES = 8
EXP_SHIFT = -25.0

CNB = 16           # blocks per chunk (= half a head)
N_CHUNKS = (H * NB) // CNB
QK_COLS = 2 * CNB * W      # 4096: qT | kT
V_COLS = CNB * (D + 1)     # 2064: v with ones column baked in per block
EG = 8             # blocks per exp group (two PSUM banks of scores)
PG = 3             # blocks per PV group (3*129 <= 512 psum cols)
ACT_GROUP = None   # ACT normalize offload disabled: cross-engine waits
                   # head-of-line block the in-order scalar queue


def build_bass(num_devices: int = N_CORES) -> bass.Bass:
    f16 = mybir.dt.float16
    bf16 = mybir.dt.bfloat16
    f32 = mybir.dt.float32
    nc = bacc.Bacc(
        "TRN2", target_bir_lowering=False, debug=False, num_devices=num_devices
    )
    xqk = nc.dram_tensor(
        "xqk", (N_CHUNKS * 128, QK_COLS), f16, kind="ExternalInput"
    ).ap()
    xv = nc.dram_tensor(
        "xv", (N_CHUNKS * 128, V_COLS), f16, kind="ExternalInput"
    ).ap()
    o = nc.dram_tensor(
        "out", (N_CHUNKS * 128, CNB * D), f16, kind="ExternalOutput"
    ).ap()

    with tile.TileContext(nc) as tc:
        with (
            tc.tile_pool(name="pqk", bufs=9) as pqk,
            tc.tile_pool(name="pv", bufs=14) as pv,
            tc.tile_pool(name="po", bufs=6) as po,
            tc.tile_pool(name="probs", bufs=8) as probs,
            tc.tile_pool(name="small", bufs=8) as small,
            tc.tile_pool(name="const", bufs=1) as const,
            tc.tile_pool(name="ps_s", bufs=2, space="PSUM") as ps_s,
            tc.tile_pool(name="ps_o", bufs=4, space="PSUM") as ps_o,
        ):
            exp_bias = const.tile([128, 1], f32)
            nc.gpsimd.memset(exp_bias, EXP_SHIFT)

            def score_exp(cc):
                """Input DMAs + score matmuls + batched exp for chunk cc.
                Returns (vt, pTs) needed by the PV phase."""
                qk = pqk.tile([128, QK_COLS], f16, tag="qk")
                nc.sync.dma_start(out=qk, in_=xqk[cc * 128 : (cc + 1) * 128])
                vt = pv.tile([128, V_COLS], f16, tag="vt")
                nc.sync.dma_start(out=vt, in_=xv[cc * 128 : (cc + 1) * 128])
                pTs = []
                for g in range(CNB // EG):
                    sT = ps_s.tile([128, EG * W], f32, tag="sT")
                    for i in range(EG):
                        n = g * EG + i
                        # sT[u, w] = k[u,:] . q[w,:]
                        nc.tensor.matmul(
                            sT[:, i * W : (i + 1) * W],
                            qk[:, (CNB + n) * W : (CNB + n + 1) * W],
                            qk[:, n * W : (n + 1) * W],
                        )
                    pT = probs.tile([128, EG * W], bf16, tag="pT")
                    nc.scalar.activation(
                        pT, sT, mybir.ActivationFunctionType.Exp,
                        bias=exp_bias, scale=1.0,
                    )
                    pTs.append(pT)
                return vt, pTs

            def pv_norm_out(cc, vt, pTs):
                """PV matmuls (PG blocks per PSUM bank), normalize straight
                out of PSUM (reciprocal of the denominator column, then a
                broadcast tensor_mul on DVE -- or per-block ACT scale-copies
                for one group per chunk to balance the queues), output DMA."""
                oh = po.tile([128, CNB, D], f16, tag="oh")
                for gi, n0 in enumerate(range(0, CNB, PG)):
                    nn = min(PG, CNB - n0)
                    o_ps = ps_o.tile([128, PG, D + 1], f32, tag="o_ps")
                    for j in range(nn):
                        n = n0 + j
                        pT = pTs[n // EG][:, (n % EG) * W : (n % EG + 1) * W]
                        # out[w, 0:D] = probs @ vb ; out[w, D] = exp row sum
                        nc.tensor.matmul(
                            o_ps[:, j, :],
                            pT,
                            vt[:, n * (D + 1) : (n + 1) * (D + 1)],
                        )
                    r = small.tile([128, PG, 1], f32, tag="r")
                    nc.vector.reciprocal(
                        r[:, 0:nn, :], o_ps[:, 0:nn, D : D + 1]
                    )
                    if gi == ACT_GROUP:
                        # per-block fused scale-copy on ACT (scale is a
                        # per-partition vector) to offload DVE
                        for j in range(nn):
                            nc.scalar.mul(
                                oh[:, n0 + j, :], o_ps[:, j, 0:D], r[:, j, :]
                            )
                    else:
                        nc.vector.tensor_mul(
                            oh[:, n0 : n0 + nn, :],
                            o_ps[:, 0:nn, 0:D],
                            r[:, 0:nn, :].broadcast_to([128, nn, D]),
                        )
                nc.gpsimd.dma_start(
                    out=o[cc * 128 : (cc + 1) * 128], in_=oh
                )

            # software pipelining: emit chunk cc's scores before chunk
            # cc-1's PV phase
            prev = None
            for cc in range(N_CHUNKS):
                cur = (cc, *score_exp(cc))
                if prev is not None:
                    pv_norm_out(*prev)
                prev = cur
            pv_norm_out(*prev)

    nc.compile()
    return nc


_nc_cache = None


def _get_nc() -> bass.Bass:
    global _nc_cache
    if _nc_cache is None:
        _nc_cache = build_bass()
    return _nc_cache


def _pack_inputs(q, k, v):
    """Pack one batch's q,k,v [H,L,D] fp32 into device layouts:
    xqk [N_CHUNKS*128, QK_COLS] (qT | kT per chunk) and
    xv [N_CHUNKS*128, V_COLS] (v with ones column per block), both fp16."""
    xqk = np.empty((N_CHUNKS, 128, QK_COLS), dtype=np.float16)
    half = CNB * W
    # (cc, p, w, d) -> (cc, d, p, w)
    xqk[:, :, :half] = (
        q.reshape(N_CHUNKS, CNB, W, D).transpose(0, 3, 1, 2).reshape(N_CHUNKS, D, half)
    )
    xqk[:, :, half:] = (
        k.reshape(N_CHUNKS, CNB, W, D).transpose(0, 3, 1, 2).reshape(N_CHUNKS, D, half)
    )
    xv = np.empty((N_CHUNKS, 128, CNB, D + 1), dtype=np.float16)
    # (cc, p, u, d) -> (cc, u, p, d)
    xv[:, :, :, :D] = v.reshape(N_CHUNKS, CNB, W, D).transpose(0, 2, 1, 3)
    xv[:, :, :, D] = 1.0
    return (
        xqk.reshape(N_CHUNKS * 128, QK_COLS),
        xv.reshape(N_CHUNKS * 128, V_COLS),
    )


def _prepare_in_maps(q, k, v):
    q = np.asarray(q, dtype=np.float32)
    k = np.asarray(k, dtype=np.float32)
    v = np.asarray(v, dtype=np.float32)
    assert q.shape == (B, H, L, D), q.shape
    maps = []
    for b in range(B):
        xqk, xv = _pack_inputs(q[b], k[b], v[b])
        maps.append({"xqk": xqk, "xv": xv})
    return maps


def _unpack_out(o: np.ndarray) -> np.ndarray:
    """[N_CHUNKS*128, CNB*D] fp16 -> [H, L, D] fp32."""
    return (
        o.reshape(N_CHUNKS, W, CNB, D).transpose(0, 2, 1, 3).reshape(H, L, D)
    ).astype(np.float32)


def kernel(**inputs: np.ndarray) -> np.ndarray:
    nc = _get_nc()
    in_maps = _prepare_in_maps(inputs["q"], inputs["k"], inputs["v"])
    res = bass_utils.run_bass_kernel_spmd(nc, in_maps, core_ids=list(range(N_CORES)))
    return np.stack(
        [_unpack_out(np.asarray(res.results[b]["out"])) for b in range(B)], axis=0
    )
